# revision 25
# baseline (speedup 1.0000x reference)
"""Trainium2 Bass kernel for nn_CausalSelfAttention_2783138808334.

B=8, T=1024, C=64, n_head=1. Data-parallel over batch: one batch per
NeuronCore across 8 cores (weights/tables replicated), gathered on the host.

v8: fp8 skew round trip + host-folded output projection.

Raw (pre-scaled, 1/sqrt(C) folded into Wq/bq on the host) scores are
evacuated from PSUM straight to float8e4 staging -- quantization error on
|score| <~ 0.5 is ~1e-3 absolute, i.e. ~0.1% after exp -- halving the DRAM
skew traffic that paces the kernel's middle phase.  The two evac streams run
on different engines (DVE: qe, ACT: a1-reversed) so writes are paced at half
the single-engine rate.

Skew geometry (per tile, one DRAM row per t at pitch P1, fp8):
  row = [qe-rev (Wd) | -200 gap | a1-rev (Wd) | -200 gap]
read back with partition step P1-1 gives the sheared copies in natural
orientation with -200 landing exactly on the causal-mask region (exp -> 0).
Post-trip per tile: s1 = natural + sheared (DVE tensor_add, fp8 in / bf16
out), exp on ACT.

Wproj is folded into v and embv on the host (y = E_s@(v Wp^T) +
E_u@(embv Wp^T)), so the device has no output projection; a ones column in
v' (zeros in embv') makes the value matmuls emit Z = sum_s exp as row 64.
The device ships unnormalized y^T plus Z; the host divides and adds bproj.

E / AU are transposed 128x128-blockwise on the TensorEngine; value matmuls
accumulate into two PSUM banks.  Tiles are processed in order
3,7,6,5,4,2,1,0: tile 3 needs only the first qk-projection chunk so score
matmuls start early; the big tiles' round trips overlap the evacuation
stream; the tiny tiles 1,0 make a short tail; the k-th value term fires at
stage3(k) (k=3 rides with k=4), spreading the PE work.  Score matmul pairs
run concurrently on PE row-groups (0,0)/(64,0) via tile_position.
"""
import numpy as np
import ml_dtypes

import concourse.bass as bass
import concourse.bacc as bacc
import concourse.mybir as mybir
from concourse import masks
from concourse.ap import AP

F32 = mybir.dt.float32
BF = mybir.dt.bfloat16
F8 = mybir.dt.float8e4
T = 1024
C = 64
NT = 8
P1 = 4096       # skew scratch row pitch (elements)
N_WARM = 4      # PE warm-up matmuls
EXP = mybir.ActivationFunctionType.Exp
MULT = mybir.AluOpType.mult
ADD = mybir.AluOpType.add


def rev_free(ap):
    """Reverse the (contiguous) free dim of a 2D AP."""
    (ps, pc), (fs, fc) = ap.ap
    assert fs == 1, ap.ap
    return AP(ap.tensor, ap.offset + (fc - 1), [[ps, pc], [-1, fc]])


def mm_chunks(lo, hi, step=512):
    a = lo
    while a < hi:
        b = min(hi, (a // step + 1) * step)
        yield a, b
        a = b


def emit(nc, tc, xta_d, ekr_d, const_d, yd):
    with (
        tc.tile_pool(name="const", bufs=1) as cp,
        tc.tile_pool(name="work", bufs=1) as wp,
        tc.tile_pool(name="psum", bufs=1, space="PSUM") as pp,
        tc.tile_pool(name="dram", bufs=1, space="DRAM") as dp,
    ):
        QAD = dp.tile([T + 1, P1], F8, name="QAD").tensor

        # ---- loads ----
        XTA = cp.tile([65, T], BF)      # [x.T ; ones]
        KNE = cp.tile([128, T], BF)     # rows 0:64 k.T (natural); 64:128 embk.T-rev
        CONST = cp.tile([128, 712], BF)
        FILLC = cp.tile([128, 128], BF)     # warm-up fodder
        nc.vector.memset(FILLC, 0.0)
        FILL8 = cp.tile([128, 1024], F8)    # gap prefill source (mask value)
        nc.vector.memset(FILL8, -200.0)
        ZROW = cp.tile([1, 512], BF)
        nc.vector.memset(ZROW, 0.0)
        # XTA first (it gates qk) on sync; prefills next on the SAME queue as
        # the skew reads (sync) so FIFO order guarantees prefill-before-read.
        nc.sync.dma_start(out=XTA[:, 0:512], in_=xta_d[:, 0:512])
        nc.sync.dma_start(out=XTA[:, 512:1024], in_=xta_d[:, 512:1024])
        # weights (needed by qk proj) first, then ekr (scores), then embv'.
        nc.scalar.dma_start(out=CONST[:, 520:712], in_=const_d[:, 520:712])
        nc.scalar.dma_start(out=KNE[64:128, 512:1024], in_=ekr_d[:, 512:1024])
        nc.scalar.dma_start(out=KNE[64:128, 0:512], in_=ekr_d[:, 0:512])
        nc.scalar.dma_start(out=CONST[:, 0:520], in_=const_d[:, 0:520])
        EMBV = CONST[:, 0:520]          # embv@Wp.T row-packed [p, 65n+c], col 64 = 0
        WVA = CONST[0:65, 520:584]      # [(Wp Wv).T ; (Wp bv)]
        WQKB = CONST[0:65, 584:712]     # [[Wq.T | Wk.T] ; [bq | bk]] (q pre-scaled)
        # zero prefill of the two gap regions of every tile's rows
        nc.sync.dma_start(
            out=AP(QAD, P1 + 128, [[P1, 128], [128 * P1 + 128, NT], [1, 128]]),
            in_=FILL8.rearrange("p (b c) -> p b c", b=NT))
        nc.sync.dma_start(
            out=AP(QAD, P1 + 384, [[P1, 128], [128 * P1 + 256, NT], [1, 128]]),
            in_=FILL8.rearrange("p (b c) -> p b c", b=NT))

        identb = cp.tile([128, 128], BF)
        masks.make_identity(nc, identb)

        # ---- PE warm-up burst (garbage matmuls, result never read) ----
        wu = pp.tile([128, 128], F32, tag="A", bufs=4, name="wu")
        for _ in range(N_WARM):
            nc.tensor.matmul(wu[:, :], FILLC[:, :], FILLC[:, :],
                             start=True, stop=True)

        # ---- qk projection: [q.T ; k.T] = [Wq.T|Wk.T ; bq|bk].T @ [x.T ; 1]
        QT2 = cp.tile([128, T], BF)     # q.T duplicated in both halves
        for a, b in mm_chunks(0, T):
            ps_qk = pp.tile([128, 512], F32, tag="A" if a == 0 else "B",
                            bufs=4, name="ps_qk")
            nc.tensor.matmul(ps_qk[:, 0:b - a], WQKB, XTA[:, a:b],
                             start=True, stop=True)
            nc.vector.tensor_copy(KNE[0:64, a:b], ps_qk[64:128, 0:b - a])
            nc.scalar.copy(QT2[0:64, a:b], ps_qk[0:64, 0:b - a])
            nc.vector.tensor_copy(QT2[64:128, a:b], ps_qk[0:64, 0:b - a])

        # ---- stage 1: score matmuls, fused exp evac (ACT), skew trip ----
        qa_t = {}
        au2_t = {}

        def stage1(i):
            Wd = 128 * (i + 1)
            i0 = 128 * i
            qa = cp.tile([128, 2048], F8, tag=f"qa{i}", name=f"qa{i}")
            qa_t[i] = qa
            for a, b in mm_chunks(0, Wd):
                w = b - a
                ps_a1 = pp.tile([128, 512], F32, tag="A", bufs=4, name="ps_a1")
                ps_qe = pp.tile([128, 512], F32, tag="B", bufs=4, name="ps_qe")
                nc.tensor.matmul(ps_a1[:, 0:w], QT2[0:64, i0:i0 + 128],
                                 KNE[0:64, a:b],
                                 start=True, stop=True, tile_position=(0, 0))
                nc.tensor.matmul(ps_qe[:, 0:w], QT2[64:128, i0:i0 + 128],
                                 KNE[64:128, T - Wd + a:T - Wd + b],
                                 start=True, stop=True, tile_position=(64, 0))
                # qa row = [qe-rev (Wd) | a1-rev (Wd)] raw fp8; DVE/ACT split
                nc.vector.tensor_copy(qa[:, a:b], ps_qe[:, 0:w])
                nc.scalar.copy(
                    rev_free(qa[:, 2 * Wd - b:2 * Wd - a]), ps_a1[:, 0:w])
            # one merged write per tile: [EQr | gap | EAr] at pitch P1
            nc.gpsimd.dma_start(
                out=AP(QAD, (i0 + 1) * P1,
                       [[P1, 128], [Wd + 128, 2], [1, Wd]]),
                in_=AP(qa[:, :].tensor, qa[:, :].offset,
                       [[qa[:, :].ap[0][0], 128], [Wd, 2], [1, Wd]]))
            # merged skew read: EQs = [:, 0:Wd], EAs = [:, Wd+128:2Wd+128]
            L = 2 * Wd + 128
            au2 = cp.tile([128, 2304], F8, tag=f"au2{i}", name=f"au2{i}")
            au2_t[i] = au2
            nc.sync.dma_start(
                out=au2[:, 0:L],
                in_=AP(QAD, (i0 + 1) * P1 + 127, [[P1 - 1, 128], [1, L]]))

        V = cp.tile([128, 520], BF)     # v'[128n+p, c] at [p, 65n+c]; col 64 = 1
        ETB = cp.tile([128, NT * 1024], BF, name="ETB")
        AUTB = cp.tile([128, NT * 1024], BF, name="AUTB")
        ETB3 = ETB.rearrange("p (k c) -> p k c", c=1024)
        AUTB3 = AUTB.rearrange("p (k c) -> p k c", c=1024)
        ps_y1 = pp.tile([65, 512], F32, tag="A", bufs=4, name="ps_y1")
        ps_y0 = pp.tile([65, 512], F32, tag="B", bufs=4, name="ps_y0")
        enau_t = {}

        def stage2(i):
            Wd = 128 * (i + 1)
            qa = qa_t[i]
            au2 = au2_t[i]
            enau = cp.tile([128, 2048], BF, tag=f"enau{i}", name=f"enau{i}")
            enau_t[i] = enau
            s1 = wp.tile([128, 2048], BF, tag="s1", bufs=3)
            nc.vector.tensor_add(s1[:, 0:Wd], rev_free(qa[:, Wd:2 * Wd]),
                                 au2[:, 0:Wd])
            nc.vector.tensor_add(s1[:, 1024:1024 + Wd], rev_free(qa[:, 0:Wd]),
                                 au2[:, Wd + 128:2 * Wd + 128])
            nc.scalar.activation(enau[:, 0:Wd], s1[:, 0:Wd], EXP)
            nc.scalar.activation(enau[:, 1024:1024 + Wd],
                                 s1[:, 1024:1024 + Wd], EXP)

        def stage3(i):
            i0 = 128 * i
            enau = enau_t[i]
            for half, dst3 in ((0, ETB3), (1, AUTB3)):
                for g in range(0, i + 1, 8):
                    gsz = min(8, i + 1 - g)
                    ps_t4 = pp.tile([128, 1024], BF, tag="B", bufs=4,
                                    name="ps_t4")
                    for j in range(gsz):
                        k = g + j
                        nc.tensor.matmul(
                            ps_t4[:, 128 * j:128 * (j + 1)],
                            enau[:, 1024 * half + 128 * k:1024 * half + 128 * (k + 1)],
                            identb[:, :], is_transpose=True,
                            start=(j == 0), stop=(j == gsz - 1))
                    # interleave copy engines across the stage3 sequence
                    cpy = nc.vector.tensor_copy if i in (3, 6, 4, 1) \
                        else nc.scalar.copy
                    cpy(dst3[:, g:g + gsz, i0:i0 + 128],
                        ps_t4[:, 0:128 * gsz].rearrange("p (k c) -> p k c", c=128))
            # value terms: with order 3,7,6,5,4,2,1,0 the k-th value term's
            # t-range completes exactly at stage3(k) (k=3 rides with k=4)
            for k in ([4, 3] if i == 4 else [i] if i != 3 else []):
                k0 = 128 * k
                ta = max(512, k0)
                nc.tensor.matmul(ps_y1[:, ta - 512:512], V[:, 65 * k:65 * (k + 1)],
                                 ETB[:, 1024 * k + ta:1024 * k + T],
                                 start=False, stop=False)
                nc.tensor.matmul(ps_y1[:, ta - 512:512],
                                 EMBV[:, 65 * k:65 * (k + 1)],
                                 AUTB[:, 1024 * k + ta:1024 * k + T],
                                 start=False, stop=(k == 0))
                if k0 < 512:
                    nc.tensor.matmul(ps_y0[:, k0:512], V[:, 65 * k:65 * (k + 1)],
                                     ETB[:, 1024 * k + k0:1024 * k + 512],
                                     start=False, stop=False)
                    nc.tensor.matmul(ps_y0[:, k0:512],
                                     EMBV[:, 65 * k:65 * (k + 1)],
                                     AUTB[:, 1024 * k + k0:1024 * k + 512],
                                     start=False, stop=(k == 0))

        # ---- driver: stage1 all, v-projs, then lag-1 stage2/stage3 ----
        ORDER = [3, 7, 6, 5, 4, 2, 1, 0]
        for i in ORDER:
            stage1(i)
        # v' projection (PE filler during the first tiles' skew round trips)
        for n in range(NT):
            ps_v = pp.tile([128, C], F32, tag="A", bufs=4, name="ps_v")
            nc.tensor.matmul(ps_v[:, :], XTA[:, 128 * n:128 * (n + 1)], WVA,
                             start=True, stop=True)
            nc.vector.tensor_copy(V[:, 65 * n:65 * n + 64], ps_v[:, :])
        nc.vector.memset(AP(V[:, :].tensor, V[:, :].offset + 64,
                            [[V[:, :].ap[0][0], 128], [65, NT]]), 1.0)
        nc.tensor.matmul(ps_y1[:, :], ZROW[:, 0:65], ZROW[:, :],
                         start=True, stop=False)
        nc.tensor.matmul(ps_y0[:, :], ZROW[:, 0:65], ZROW[:, :],
                         start=True, stop=False)
        prev = None
        for i in ORDER:
            stage2(i)
            if prev is not None:
                stage3(prev)
            prev = i
        stage3(0)

        # ---- ship unnormalized y^T plus Z row; host divides by Z ----
        YT = cp.tile([65, T], BF)
        nc.vector.tensor_copy(YT[:, 0:512], ps_y0[:, :])
        nc.scalar.copy(YT[:, 512:1024], ps_y1[:, :])
        nc.sync.dma_start(out=yd[:, :], in_=YT[:, :])


_NC_CACHE = None


def _build():
    global _NC_CACHE
    if _NC_CACHE is not None:
        return _NC_CACHE
    nc = bacc.Bacc("TRN2", target_bir_lowering=False, debug=False)
    xta_d = nc.dram_tensor("xta", [65, T], BF, kind="ExternalInput")
    ekr_d = nc.dram_tensor("ekr", [C, T], BF, kind="ExternalInput")
    const_d = nc.dram_tensor("cpack", [128, 712], BF, kind="ExternalInput")
    yd = nc.dram_tensor("y", [65, T], BF, kind="ExternalOutput")
    from concourse.tile import TileContext
    with TileContext(nc) as tc:
        emit(nc, tc, xta_d.ap(), ekr_d.ap(), const_d.ap(), yd.ap())
    nc.compile()
    _NC_CACHE = nc
    return nc


def _prep(inputs):
    """Host-side packing of all device inputs (bf16, pre-transposed).

    1/sqrt(C) is folded into Wq/bq; Wproj is folded into Wv/bv and embv.
    """
    bf = ml_dtypes.bfloat16
    x = np.asarray(inputs["x"], dtype=np.float32)
    Wqkv = np.asarray(inputs["Wqkv"], dtype=np.float32)
    bqkv = np.asarray(inputs["bqkv"], dtype=np.float32)
    embk = np.asarray(inputs["embk"], dtype=np.float32)
    embv = np.asarray(inputs["embv"], dtype=np.float32)
    Wproj = np.asarray(inputs["Wproj"], dtype=np.float32)
    scale = 1.0 / np.sqrt(np.float32(x.shape[2]))

    B = x.shape[0]
    xta = np.empty((B, 65, T), dtype=bf)
    for b in range(B):
        xta[b, 0:64] = x[b].T.astype(bf)
        xta[b, 64] = 1.0
    ekr = np.ascontiguousarray(embk.T[:, ::-1]).astype(bf)

    Wv = Wqkv[128:192, :]               # v = x @ Wv.T + bv
    bv = bqkv[128:192]
    Wvp = Wproj @ Wv                    # v' = v @ Wproj.T = x @ Wvp.T + bvp
    bvp = Wproj @ bv
    embvp = embv @ Wproj.T              # [T, C]

    const = np.zeros((128, 712), dtype=bf)
    ev = embvp.reshape(8, 128, 64).transpose(1, 0, 2)          # [128, 8, 64]
    evp = np.zeros((128, 8, 65), dtype=np.float32)
    evp[:, :, 0:64] = ev
    const[:, 0:520] = evp.reshape(128, 520).astype(bf)
    const[0:64, 520:584] = Wvp.T.astype(bf)
    const[64, 520:584] = bvp.astype(bf)
    const[0:64, 584:648] = (Wqkv[0:64, :] * scale).T.astype(bf)
    const[64, 584:648] = (bqkv[0:64] * scale).astype(bf)
    const[0:64, 648:712] = Wqkv[64:128, :].T.astype(bf)
    const[64, 648:712] = bqkv[64:128].astype(bf)
    return xta, np.ascontiguousarray(ekr), np.ascontiguousarray(const)


def run_spmd(inputs, **kwargs):
    from concourse.bass_utils import run_bass_kernel_spmd
    x = np.asarray(inputs["x"], dtype=np.float32)
    B = x.shape[0]
    nc = _build()
    xta, ekr, const = _prep(inputs)
    in_maps = [dict(xta=np.ascontiguousarray(xta[b]), ekr=ekr, cpack=const)
               for b in range(B)]
    res = run_bass_kernel_spmd(nc, in_maps, core_ids=list(range(B)), **kwargs)
    bproj = np.asarray(inputs["bproj"], dtype=np.float32)
    ys = []
    for r in res.results:
        yt = np.asarray(r["y"], dtype=np.float32)  # [65, T]: 0:64 = y^T, 64 = Z
        ys.append(yt[0:64].T / yt[64][:, None] + bproj[None, :])
    return np.stack(ys, axis=0), res


def kernel(**inputs):
    y, _ = run_spmd(inputs)
    return y


# revision 26
# speedup vs baseline: 1.0305x; 1.0305x over previous
"""Trainium2 Bass kernel for nn_CausalSelfAttention_2783138808334.

B=8, T=1024, C=64, n_head=1. Data-parallel over batch: one batch per
NeuronCore across 8 cores (weights/tables replicated), gathered on the host.

v8: fp8 skew round trip + host-folded output projection.

Raw (pre-scaled, 1/sqrt(C) folded into Wq/bq on the host) scores are
evacuated from PSUM straight to float8e4 staging -- quantization error on
|score| <~ 0.5 is ~1e-3 absolute, i.e. ~0.1% after exp -- halving the DRAM
skew traffic that paces the kernel's middle phase.  The two evac streams run
on different engines (DVE: qe, ACT: a1-reversed) so writes are paced at half
the single-engine rate.

Skew geometry (per tile, one DRAM row per t at pitch P1, fp8):
  row = [qe-rev (Wd) | -200 gap | a1-rev (Wd) | -200 gap]
read back with partition step P1-1 gives the sheared copies in natural
orientation with -200 landing exactly on the causal-mask region (exp -> 0).
Post-trip per tile: s1 = natural + sheared (DVE tensor_add, fp8 in / bf16
out), exp on ACT.

Wproj is folded into v and embv on the host (y = E_s@(v Wp^T) +
E_u@(embv Wp^T)), so the device has no output projection; a ones column in
v' (zeros in embv') makes the value matmuls emit Z = sum_s exp as row 64.
The device ships unnormalized y^T plus Z; the host divides and adds bproj.

E / AU are transposed 128x128-blockwise on the TensorEngine; value matmuls
accumulate into two PSUM banks.  Tiles are processed in order
3,7,6,5,4,2,1,0: tile 3 needs only the first qk-projection chunk so score
matmuls start early; the big tiles' round trips overlap the evacuation
stream; the tiny tiles 1,0 make a short tail; the k-th value term fires at
stage3(k) (k=3 rides with k=4), spreading the PE work.  Score matmul pairs
run concurrently on PE row-groups (0,0)/(64,0) via tile_position.
"""
import numpy as np
import ml_dtypes

import concourse.bass as bass
import concourse.bacc as bacc
import concourse.mybir as mybir
from concourse import masks
from concourse.ap import AP

F32 = mybir.dt.float32
BF = mybir.dt.bfloat16
F8 = mybir.dt.float8e4
T = 1024
C = 64
NT = 8
P1 = 4096       # skew scratch row pitch (elements)
N_WARM = 4      # PE warm-up matmuls
EXP = mybir.ActivationFunctionType.Exp
MULT = mybir.AluOpType.mult
ADD = mybir.AluOpType.add


def rev_free(ap):
    """Reverse the (contiguous) free dim of a 2D AP."""
    (ps, pc), (fs, fc) = ap.ap
    assert fs == 1, ap.ap
    return AP(ap.tensor, ap.offset + (fc - 1), [[ps, pc], [-1, fc]])


def mm_chunks(lo, hi, step=512):
    a = lo
    while a < hi:
        b = min(hi, (a // step + 1) * step)
        yield a, b
        a = b


def emit(nc, tc, xta_d, ekr_d, const_d, yd):
    with (
        tc.tile_pool(name="const", bufs=1) as cp,
        tc.tile_pool(name="work", bufs=1) as wp,
        tc.tile_pool(name="psum", bufs=1, space="PSUM") as pp,
        tc.tile_pool(name="dram", bufs=1, space="DRAM") as dp,
    ):
        QAD = dp.tile([T + 1, P1], F8, name="QAD").tensor
        P2 = 2048
        QD2 = dp.tile([T + 1, P2], BF, name="QD2").tensor

        # ---- loads ----
        XTA = cp.tile([65, T], BF)      # [x.T ; ones]
        KNE = cp.tile([128, T], BF)     # rows 0:64 k.T (natural); 64:128 embk.T-rev
        CONST = cp.tile([128, 712], BF)
        FILLC = cp.tile([128, 512], BF)     # warm-up fodder + bf16 zero prefill
        nc.vector.memset(FILLC, 0.0)
        FILL8 = cp.tile([128, 1024], F8)    # gap prefill source (mask value)
        nc.vector.memset(FILL8, -200.0)
        ZROW = cp.tile([1, 512], BF)
        nc.vector.memset(ZROW, 0.0)
        # XTA first (it gates qk) on sync; prefills next on the SAME queue as
        # the skew reads (sync) so FIFO order guarantees prefill-before-read.
        nc.sync.dma_start(out=XTA[:, 0:512], in_=xta_d[:, 0:512])
        nc.sync.dma_start(out=XTA[:, 512:1024], in_=xta_d[:, 512:1024])
        # weights (needed by qk proj) first, then ekr (scores), then embv'.
        nc.scalar.dma_start(out=CONST[:, 520:712], in_=const_d[:, 520:712])
        nc.scalar.dma_start(out=KNE[64:128, 512:1024], in_=ekr_d[:, 512:1024])
        nc.scalar.dma_start(out=KNE[64:128, 0:512], in_=ekr_d[:, 0:512])
        nc.scalar.dma_start(out=CONST[:, 0:520], in_=const_d[:, 0:520])
        EMBV = CONST[:, 0:520]          # embv@Wp.T row-packed [p, 65n+c], col 64 = 0
        WVA = CONST[0:65, 520:584]      # [(Wp Wv).T ; (Wp bv)]
        WQKB = CONST[0:65, 584:712]     # [[Wq.T | Wk.T] ; [bq | bk]] (q pre-scaled)
        # zero prefill of the two gap regions of every tile's rows
        nc.sync.dma_start(
            out=AP(QAD, P1 + 128, [[P1, 128], [128 * P1 + 128, NT], [1, 128]]),
            in_=FILL8.rearrange("p (b c) -> p b c", b=NT))
        nc.sync.dma_start(
            out=AP(QAD, P1 + 384, [[P1, 128], [128 * P1 + 256, NT], [1, 128]]),
            in_=FILL8.rearrange("p (b c) -> p b c", b=NT))
        nc.sync.dma_start(
            out=AP(QD2, P2 + 128, [[P2, 128], [128 * P2 + 128, 4], [1, 128]]),
            in_=FILLC.rearrange("p (b c) -> p b c", b=4))
        nc.sync.dma_start(
            out=AP(QD2, P2 + 384, [[P2, 128], [128 * P2 + 256, 4], [1, 128]]),
            in_=FILLC.rearrange("p (b c) -> p b c", b=4))

        identb = cp.tile([128, 128], BF)
        masks.make_identity(nc, identb)

        # ---- PE warm-up burst (garbage matmuls, result never read) ----
        wu = pp.tile([128, 128], F32, tag="A", bufs=4, name="wu")
        for _ in range(N_WARM):
            nc.tensor.matmul(wu[:, :], FILLC[:, 0:128], FILLC[:, 0:128],
                             start=True, stop=True)

        # ---- qk projection: [q.T ; k.T] = [Wq.T|Wk.T ; bq|bk].T @ [x.T ; 1]
        QT2 = cp.tile([128, T], BF)     # q.T duplicated in both halves
        for a, b in mm_chunks(0, T):
            ps_qk = pp.tile([128, 512], F32, tag="A" if a == 0 else "B",
                            bufs=4, name="ps_qk")
            nc.tensor.matmul(ps_qk[:, 0:b - a], WQKB, XTA[:, a:b],
                             start=True, stop=True)
            nc.vector.tensor_copy(KNE[0:64, a:b], ps_qk[64:128, 0:b - a])
            nc.scalar.copy(QT2[0:64, a:b], ps_qk[0:64, 0:b - a])
            nc.vector.tensor_copy(QT2[64:128, a:b], ps_qk[0:64, 0:b - a])

        # ---- stage 1: score matmuls, fused exp evac (ACT), skew trip ----
        qa_t = {}
        au2_t = {}

        def stage1(i):
            Wd = 128 * (i + 1)
            i0 = 128 * i
            small = i < 4          # exp'd bf16 trip; big tiles: raw fp8 trip
            dt = BF if small else F8
            QD, PD = (QD2, 2048) if small else (QAD, P1)
            qa = cp.tile([128, 2048], dt, tag=f"qa{i}", name=f"qa{i}")
            qa_t[i] = qa
            for a, b in mm_chunks(0, Wd):
                w = b - a
                ps_a1 = pp.tile([128, 512], F32, tag="A", bufs=4, name="ps_a1")
                ps_qe = pp.tile([128, 512], F32, tag="B", bufs=4, name="ps_qe")
                nc.tensor.matmul(ps_a1[:, 0:w], QT2[0:64, i0:i0 + 128],
                                 KNE[0:64, a:b],
                                 start=True, stop=True, tile_position=(0, 0))
                nc.tensor.matmul(ps_qe[:, 0:w], QT2[64:128, i0:i0 + 128],
                                 KNE[64:128, T - Wd + a:T - Wd + b],
                                 start=True, stop=True, tile_position=(64, 0))
                if small:
                    # exp'd staging: products post-trip, mask = 0.0 gaps
                    nc.scalar.activation(qa[:, a:b], ps_qe[:, 0:w], EXP)
                    nc.scalar.activation(
                        rev_free(qa[:, 2 * Wd - b:2 * Wd - a]), ps_a1[:, 0:w],
                        EXP)
                else:
                    # raw fp8 staging; DVE/ACT split
                    nc.vector.tensor_copy(qa[:, a:b], ps_qe[:, 0:w])
                    nc.scalar.copy(
                        rev_free(qa[:, 2 * Wd - b:2 * Wd - a]), ps_a1[:, 0:w])
            # one merged write per tile: [EQr | gap | EAr] at pitch PD
            nc.gpsimd.dma_start(
                out=AP(QD, (i0 + 1) * PD,
                       [[PD, 128], [Wd + 128, 2], [1, Wd]]),
                in_=AP(qa[:, :].tensor, qa[:, :].offset,
                       [[qa[:, :].ap[0][0], 128], [Wd, 2], [1, Wd]]))
            # merged skew read: EQs = [:, 0:Wd], EAs = [:, Wd+128:2Wd+128]
            L = 2 * Wd + 128
            au2 = cp.tile([128, 2304], dt, tag=f"au2{i}", name=f"au2{i}")
            au2_t[i] = au2
            nc.sync.dma_start(
                out=au2[:, 0:L],
                in_=AP(QD, (i0 + 1) * PD + 127, [[PD - 1, 128], [1, L]]))

        V = cp.tile([128, 520], BF)     # v'[128n+p, c] at [p, 65n+c]; col 64 = 1
        ETB = cp.tile([128, NT * 1024], BF, name="ETB")
        AUTB = cp.tile([128, NT * 1024], BF, name="AUTB")
        ETB3 = ETB.rearrange("p (k c) -> p k c", c=1024)
        AUTB3 = AUTB.rearrange("p (k c) -> p k c", c=1024)
        ps_y1 = pp.tile([65, 512], F32, tag="A", bufs=4, name="ps_y1")
        ps_y0 = pp.tile([65, 512], F32, tag="B", bufs=4, name="ps_y0")
        enau_t = {}

        def stage2(i):
            Wd = 128 * (i + 1)
            qa = qa_t[i]
            au2 = au2_t[i]
            enau = cp.tile([128, 2048], BF, tag=f"enau{i}", name=f"enau{i}")
            enau_t[i] = enau
            if i < 4:
                nc.vector.tensor_mul(enau[:, 0:Wd],
                                     rev_free(qa[:, Wd:2 * Wd]), au2[:, 0:Wd])
                nc.vector.tensor_mul(enau[:, 1024:1024 + Wd],
                                     rev_free(qa[:, 0:Wd]),
                                     au2[:, Wd + 128:2 * Wd + 128])
            else:
                s1 = wp.tile([128, 2048], BF, tag="s1", bufs=3)
                nc.vector.tensor_add(s1[:, 0:Wd], rev_free(qa[:, Wd:2 * Wd]),
                                     au2[:, 0:Wd])
                nc.vector.tensor_add(s1[:, 1024:1024 + Wd],
                                     rev_free(qa[:, 0:Wd]),
                                     au2[:, Wd + 128:2 * Wd + 128])
                nc.scalar.activation(enau[:, 0:Wd], s1[:, 0:Wd], EXP)
                nc.scalar.activation(enau[:, 1024:1024 + Wd],
                                     s1[:, 1024:1024 + Wd], EXP)

        def stage3(i):
            i0 = 128 * i
            enau = enau_t[i]
            for half, dst3 in ((0, ETB3), (1, AUTB3)):
                for g in range(0, i + 1, 8):
                    gsz = min(8, i + 1 - g)
                    ps_t4 = pp.tile([128, 1024], BF, tag="B", bufs=4,
                                    name="ps_t4")
                    for j in range(gsz):
                        k = g + j
                        nc.tensor.matmul(
                            ps_t4[:, 128 * j:128 * (j + 1)],
                            enau[:, 1024 * half + 128 * k:1024 * half + 128 * (k + 1)],
                            identb[:, :], is_transpose=True,
                            start=(j == 0), stop=(j == gsz - 1))
                    # interleave copy engines across the stage3 sequence
                    cpy = nc.vector.tensor_copy if i in (3, 6, 4, 1) \
                        else nc.scalar.copy
                    cpy(dst3[:, g:g + gsz, i0:i0 + 128],
                        ps_t4[:, 0:128 * gsz].rearrange("p (k c) -> p k c", c=128))
            # value terms: with order 3,7,6,5,4,2,1,0 the k-th value term's
            # t-range completes exactly at stage3(k) (k=3 rides with k=4)
            for k in ([4, 3] if i == 4 else [i] if i != 3 else []):
                k0 = 128 * k
                ta = max(512, k0)
                nc.tensor.matmul(ps_y1[:, ta - 512:512], V[:, 65 * k:65 * (k + 1)],
                                 ETB[:, 1024 * k + ta:1024 * k + T],
                                 start=False, stop=False)
                nc.tensor.matmul(ps_y1[:, ta - 512:512],
                                 EMBV[:, 65 * k:65 * (k + 1)],
                                 AUTB[:, 1024 * k + ta:1024 * k + T],
                                 start=False, stop=(k == 0))
                if k0 < 512:
                    nc.tensor.matmul(ps_y0[:, k0:512], V[:, 65 * k:65 * (k + 1)],
                                     ETB[:, 1024 * k + k0:1024 * k + 512],
                                     start=False, stop=False)
                    nc.tensor.matmul(ps_y0[:, k0:512],
                                     EMBV[:, 65 * k:65 * (k + 1)],
                                     AUTB[:, 1024 * k + k0:1024 * k + 512],
                                     start=False, stop=(k == 0))

        # ---- driver: stage1 all, v-projs, then lag-1 stage2/stage3 ----
        ORDER = [3, 7, 6, 5, 4, 2, 1, 0]
        for i in ORDER:
            stage1(i)
        # v' projection (PE filler during the first tiles' skew round trips)
        for n in range(NT):
            ps_v = pp.tile([128, C], F32, tag="A", bufs=4, name="ps_v")
            nc.tensor.matmul(ps_v[:, :], XTA[:, 128 * n:128 * (n + 1)], WVA,
                             start=True, stop=True)
            nc.vector.tensor_copy(V[:, 65 * n:65 * n + 64], ps_v[:, :])
        nc.vector.memset(AP(V[:, :].tensor, V[:, :].offset + 64,
                            [[V[:, :].ap[0][0], 128], [65, NT]]), 1.0)
        nc.tensor.matmul(ps_y1[:, :], ZROW[:, 0:65], ZROW[:, :],
                         start=True, stop=False)
        nc.tensor.matmul(ps_y0[:, :], ZROW[:, 0:65], ZROW[:, :],
                         start=True, stop=False)
        prev = None
        for i in ORDER:
            stage2(i)
            if prev is not None:
                stage3(prev)
            prev = i
        stage3(0)

        # ---- ship unnormalized y^T plus Z row; host divides by Z ----
        YT = cp.tile([65, T], BF)
        nc.vector.tensor_copy(YT[:, 0:512], ps_y0[:, :])
        nc.scalar.copy(YT[:, 512:1024], ps_y1[:, :])
        nc.sync.dma_start(out=yd[:, :], in_=YT[:, :])


_NC_CACHE = None


def _build():
    global _NC_CACHE
    if _NC_CACHE is not None:
        return _NC_CACHE
    nc = bacc.Bacc("TRN2", target_bir_lowering=False, debug=False)
    xta_d = nc.dram_tensor("xta", [65, T], BF, kind="ExternalInput")
    ekr_d = nc.dram_tensor("ekr", [C, T], BF, kind="ExternalInput")
    const_d = nc.dram_tensor("cpack", [128, 712], BF, kind="ExternalInput")
    yd = nc.dram_tensor("y", [65, T], BF, kind="ExternalOutput")
    from concourse.tile import TileContext
    with TileContext(nc) as tc:
        emit(nc, tc, xta_d.ap(), ekr_d.ap(), const_d.ap(), yd.ap())
    nc.compile()
    _NC_CACHE = nc
    return nc


def _prep(inputs):
    """Host-side packing of all device inputs (bf16, pre-transposed).

    1/sqrt(C) is folded into Wq/bq; Wproj is folded into Wv/bv and embv.
    """
    bf = ml_dtypes.bfloat16
    x = np.asarray(inputs["x"], dtype=np.float32)
    Wqkv = np.asarray(inputs["Wqkv"], dtype=np.float32)
    bqkv = np.asarray(inputs["bqkv"], dtype=np.float32)
    embk = np.asarray(inputs["embk"], dtype=np.float32)
    embv = np.asarray(inputs["embv"], dtype=np.float32)
    Wproj = np.asarray(inputs["Wproj"], dtype=np.float32)
    scale = 1.0 / np.sqrt(np.float32(x.shape[2]))

    B = x.shape[0]
    xta = np.empty((B, 65, T), dtype=bf)
    for b in range(B):
        xta[b, 0:64] = x[b].T.astype(bf)
        xta[b, 64] = 1.0
    ekr = np.ascontiguousarray(embk.T[:, ::-1]).astype(bf)

    Wv = Wqkv[128:192, :]               # v = x @ Wv.T + bv
    bv = bqkv[128:192]
    Wvp = Wproj @ Wv                    # v' = v @ Wproj.T = x @ Wvp.T + bvp
    bvp = Wproj @ bv
    embvp = embv @ Wproj.T              # [T, C]

    const = np.zeros((128, 712), dtype=bf)
    ev = embvp.reshape(8, 128, 64).transpose(1, 0, 2)          # [128, 8, 64]
    evp = np.zeros((128, 8, 65), dtype=np.float32)
    evp[:, :, 0:64] = ev
    const[:, 0:520] = evp.reshape(128, 520).astype(bf)
    const[0:64, 520:584] = Wvp.T.astype(bf)
    const[64, 520:584] = bvp.astype(bf)
    const[0:64, 584:648] = (Wqkv[0:64, :] * scale).T.astype(bf)
    const[64, 584:648] = (bqkv[0:64] * scale).astype(bf)
    const[0:64, 648:712] = Wqkv[64:128, :].T.astype(bf)
    const[64, 648:712] = bqkv[64:128].astype(bf)
    return xta, np.ascontiguousarray(ekr), np.ascontiguousarray(const)


def run_spmd(inputs, **kwargs):
    from concourse.bass_utils import run_bass_kernel_spmd
    x = np.asarray(inputs["x"], dtype=np.float32)
    B = x.shape[0]
    nc = _build()
    xta, ekr, const = _prep(inputs)
    in_maps = [dict(xta=np.ascontiguousarray(xta[b]), ekr=ekr, cpack=const)
               for b in range(B)]
    res = run_bass_kernel_spmd(nc, in_maps, core_ids=list(range(B)), **kwargs)
    bproj = np.asarray(inputs["bproj"], dtype=np.float32)
    ys = []
    for r in res.results:
        yt = np.asarray(r["y"], dtype=np.float32)  # [65, T]: 0:64 = y^T, 64 = Z
        ys.append(yt[0:64].T / yt[64][:, None] + bproj[None, :])
    return np.stack(ys, axis=0), res


def kernel(**inputs):
    y, _ = run_spmd(inputs)
    return y
